# revision 1
# baseline (speedup 1.0000x reference)
"""Pairwise squared L2 distance (retrieval KNN) on 8 TRN2 NeuronCores.

dist[i, j] = ||x_i||^2 + ||y_j||^2 - 2 * <x_i, y_j>

Sharding: rows of x are split across the 8 cores (data-parallel over n);
y is replicated. Each core computes a [1024, 8192] slab of the distance
matrix.

The cross term x @ y^T runs as an fp16 hi/lo split GEMM (x ~ xh + xl,
y ~ yh + yl; cross = xh@yh + xh@yl + xl@yh, accumulated in fp32 PSUM),
giving ~5e-7 relative error at full PE rate (fp32/fp32r matmuls are
2-4x slower on TRN2). The norm terms ride the epilogue: ScalarE
computes -2*psum + x_sq (per-partition bias), VectorE adds a broadcast
y_sq tile (built once on-chip by gpsimd partition_broadcast, exact
fp32), and 1 MiB stores stream the result out. Column groups are the
outer loop so compute starts after the first 1 MiB of y has landed.
Inputs are laid out host-side (transposes, fp16 split, norm vectors) so
the device does no transposes.
"""

import numpy as np

import concourse.bass as bass
import concourse.mybir as mybir
import concourse.tile as tile
from concourse import bacc
from concourse.bass import ts
from concourse.bass_utils import run_bass_kernel_spmd

N, M, D = 8192, 8192, 128
NCORES = 8
SLAB = N // NCORES  # 1024 rows of x per core
P = 128  # partitions / m-chunk height
MCH = SLAB // P  # 8 m-chunks per core
NT = 512  # matmul free-dim tile (one fp32 PSUM bank)
GW = 4  # n-chunks per PSUM group (4 banks = 8 KiB/partition)
GCOLS = GW * NT  # 2048
NG = M // GCOLS  # 4 column groups
LW = 2048  # y load-chunk width
YC = M // LW  # 4 load chunks
NCH = M // NT  # 16 n-chunks

_f32 = mybir.dt.float32
_f16 = mybir.dt.float16
_IDENT = mybir.ActivationFunctionType.Identity

_compiled_nc = None


def _build():
    """Build + compile the single-core Bass program (SPMD across 8 cores)."""
    nc = bacc.Bacc(
        "TRN2",
        target_bir_lowering=False,
        debug=False,
        enable_asserts=False,
        num_devices=NCORES,
    )
    xh = nc.dram_tensor("xh", [D, SLAB], _f16, kind="ExternalInput").ap()
    xl = nc.dram_tensor("xl", [D, SLAB], _f16, kind="ExternalInput").ap()
    yh = nc.dram_tensor("yh", [D, M], _f16, kind="ExternalInput").ap()
    yl = nc.dram_tensor("yl", [D, M], _f16, kind="ExternalInput").ap()
    xsq = nc.dram_tensor("xsq", [P, MCH], _f32, kind="ExternalInput").ap()
    ysq = nc.dram_tensor("ysq", [1, M], _f32, kind="ExternalInput").ap()
    dist = nc.dram_tensor("dist", [SLAB, M], _f32, kind="ExternalOutput").ap()

    with tile.TileContext(nc) as tc:
        with (
            tc.tile_pool(name="consts", bufs=1) as cpool,
            tc.tile_pool(name="psum", bufs=2, space="PSUM") as pspool,
            tc.tile_pool(name="abuf", bufs=4) as apool,
            tc.tile_pool(name="obuf", bufs=4) as opool,
        ):
            # First-group inputs lead so the PE can start ASAP: y chunk 0 on
            # the SP ring, then x + the epilogue vectors, then the rest of y.
            yh_sb = cpool.tile([D, M], _f16)
            yl_sb = cpool.tile([D, M], _f16)
            nc.sync.dma_start(yh_sb[:, ts(0, LW)], yh[:, ts(0, LW)])
            nc.sync.dma_start(yl_sb[:, ts(0, LW)], yl[:, ts(0, LW)])
            xh_sb = cpool.tile([D, SLAB], _f16)
            nc.sync.dma_start(xh_sb[:], xh[:])
            xl_sb = cpool.tile([D, SLAB], _f16)
            nc.sync.dma_start(xl_sb[:], xl[:])
            ysq_row = cpool.tile([1, M], _f32)
            nc.sync.dma_start(ysq_row[:], ysq[:])
            xsq_sb = cpool.tile([P, MCH], _f32)
            nc.sync.dma_start(xsq_sb[:], xsq[:])
            for c in range(1, YC):
                nc.sync.dma_start(yh_sb[:, ts(c, LW)], yh[:, ts(c, LW)])
                nc.sync.dma_start(yl_sb[:, ts(c, LW)], yl[:, ts(c, LW)])

            # ysq_b[p, j] = y_sq[j], exact fp32, built on the otherwise-idle
            # GpSimd engine in group-sized chunks.
            ysq_b = cpool.tile([P, M], _f32)
            for c in range(YC):
                nc.gpsimd.partition_broadcast(
                    ysq_b[:, ts(c, LW)], ysq_row[0:1, ts(c, LW)]
                )

            def emit_block(mc, j0, w):
                """One [128, w*NT] output block: 3*w matmuls + epilogue + store."""
                xh_w = xh_sb[:, ts(mc, P)]
                xl_w = xl_sb[:, ts(mc, P)]
                cols = w * NT
                ps = pspool.tile([P, cols], _f32, tag="ps")
                # Weight-reuse order: xh held for the first 2*w matmuls,
                # then xl for w.
                for jj in range(w):
                    nc.tensor.matmul(
                        ps[:, ts(jj, NT)],
                        xh_w,
                        yh_sb[:, ts(j0 + jj, NT)],
                        start=True,
                        stop=False,
                    )
                for jj in range(w):
                    nc.tensor.matmul(
                        ps[:, ts(jj, NT)],
                        xh_w,
                        yl_sb[:, ts(j0 + jj, NT)],
                        start=False,
                        stop=False,
                    )
                for jj in range(w):
                    nc.tensor.matmul(
                        ps[:, ts(jj, NT)],
                        xl_w,
                        yh_sb[:, ts(j0 + jj, NT)],
                        start=False,
                        stop=True,
                    )
                # Epilogue: a = -2*psum + x_sq (ACT), out = a + y_sq (DVE)
                a = apool.tile([P, cols], _f32, tag="a")
                nc.scalar.activation(
                    a[:],
                    ps[:],
                    _IDENT,
                    bias=xsq_sb[:, mc : mc + 1],
                    scale=-2.0,
                )
                ot = opool.tile([P, cols], _f32, tag="ot")
                nc.vector.tensor_add(
                    ot[:], a[:], ysq_b[:, j0 * NT : j0 * NT + cols]
                )
                nc.sync.dma_start(
                    dist[ts(mc, P), j0 * NT : j0 * NT + cols], ot[:]
                )

            for g in range(NG):
                for mc in range(MCH):
                    emit_block(mc, g * GW, GW)

    nc.compile()
    return nc


def _get_nc():
    global _compiled_nc
    if _compiled_nc is None:
        _compiled_nc = _build()
    return _compiled_nc


def make_in_maps(x: np.ndarray, y: np.ndarray) -> list[dict[str, np.ndarray]]:
    x = np.asarray(x, dtype=np.float32)
    y = np.asarray(y, dtype=np.float32)
    x_sq = np.sum(x * x, axis=1, dtype=np.float32)
    y_sq = np.sum(y * y, axis=1, dtype=np.float32)

    xt = x.T  # [D, N]
    yt = y.T  # [D, M]
    xt_hi = xt.astype(np.float16)
    xt_lo = (xt - xt_hi.astype(np.float32)).astype(np.float16)
    yt_hi = np.ascontiguousarray(yt.astype(np.float16))
    yt_lo = np.ascontiguousarray((yt - yt_hi.astype(np.float32)).astype(np.float16))

    ysq_in = np.ascontiguousarray(y_sq.reshape(1, M))

    in_maps = []
    for c in range(NCORES):
        sl = slice(c * SLAB, (c + 1) * SLAB)
        # [P, MCH]: column mc holds x_sq for rows mc*128..mc*128+127
        xsq_in = np.ascontiguousarray(x_sq[sl].reshape(MCH, P).T)
        in_maps.append(
            {
                "xh": np.ascontiguousarray(xt_hi[:, sl]),
                "xl": np.ascontiguousarray(xt_lo[:, sl]),
                "yh": yt_hi,
                "yl": yt_lo,
                "xsq": xsq_in,
                "ysq": ysq_in,
            }
        )
    return in_maps


def kernel(x: np.ndarray, y: np.ndarray, **run_kwargs) -> np.ndarray:
    nc = _get_nc()
    in_maps = make_in_maps(x, y)
    res = run_bass_kernel_spmd(nc, in_maps, core_ids=list(range(NCORES)), **run_kwargs)
    out = np.concatenate([res.results[c]["dist"] for c in range(NCORES)], axis=0)
    if run_kwargs:
        kernel.last_results = res
    return out

